# revision 16
# baseline (speedup 1.0000x reference)
"""Trainium2 Bass kernel for a quantized multi-head attention block (8-core SPMD).

Sharding: core c handles batch b=c//2 and global heads [6*(c%2), 6*(c%2)+6).
Attention is head-parallel; the output projection + residual + batchnorm run
pair-redundantly over the full sequence of the core's batch (the host keeps
the rows it needs). Cross-core reductions (quant scales, BN stats) use
AllReduce; the head->channel re-shard uses a pair-local AllGather of
int8-quantized context.
"""
import sys
sys.path.insert(0, "/opt/trn_rl_repo")
import numpy as np

B, S, D = 4, 2048, 768
H, DH, HLOC, NCORES = 12, 64, 6, 8
INV_QMAX = float(np.float32(1.0) / np.float32(127.0))
MAGIC = 12582912.0  # 1.5 * 2^23: fp32 add forces round-to-nearest-even
BN_EPS = 1e-5
N_INV = float(np.float32(1.0) / np.float32(2 * B * S))  # pair-redundant BN count

_CACHE = {}


def _build():
    import concourse.bacc as bacc
    import concourse.tile as tile
    from concourse import mybir, bass_isa

    f32, bf16, i8 = mybir.dt.float32, mybir.dt.bfloat16, mybir.dt.int8
    AF, OP, AX = mybir.ActivationFunctionType, mybir.AluOpType, mybir.AxisListType

    nc = bacc.Bacc()

    xT_d = nc.dram_tensor("xT", [D, S], f32, kind="ExternalInput")            # x[b].T
    xr_d = nc.dram_tensor("xr", [128, 16, D], f32, kind="ExternalInput")      # x[b] rows tiled
    wqkvT_d = nc.dram_tensor("wqkvT", [D, 1152], f32, kind="ExternalInput")   # w slice, transposed
    woT_d = nc.dram_tensor("woT", [D, D], f32, kind="ExternalInput")          # w_out.T
    out_d = nc.dram_tensor("out", [128, 16, D], f32, kind="ExternalOutput")

    qkv_dram = nc.dram_tensor("qkv_stash", [9, 128, S], f32)
    ns_dram = nc.dram_tensor("ns_stash", [3, 2, 16, 2, 128, 1024], i8)
    RG_ALL = [list(range(NCORES))]
    RG_PAIR = [[0, 1], [2, 3], [4, 5], [6, 7]]

    with tile.TileContext(nc) as tc, \
         tc.tile_pool(name="per", bufs=1) as per, \
         tc.tile_pool(name="drampool", bufs=1, space="DRAM") as dram:

        def allreduce(src_ap, shp, tag, op=OP.max):
            bi = dram.tile(shp, f32, tag=f"ari_{tag}")
            bo = dram.tile(shp, f32, tag=f"aro_{tag}")
            nc.gpsimd.dma_start(bi[:], src_ap)
            nc.gpsimd.collective_compute("AllReduce", op, replica_groups=RG_ALL,
                                         ins=[bi.opt()], outs=[bo.opt()])
            r = per.tile(shp, f32, tag=f"arr_{tag}")
            nc.gpsimd.dma_start(r[:], bo[:])
            return r

        # persistent small tiles
        nwo = per.tile([128, 6, D], bf16, tag="nwo")
        bc1 = per.tile([128, 4], f32, tag="bc1")      # cw, cwo, s_x, 1/s_x
        bc2 = per.tile([128, 8], f32, tag="bc2")      # cq, ck, cv, s1, c
        sc_qkv = per.tile([1, 4], f32, tag="sc_qkv")  # s_q, s_k, s_v, g
        sc3 = per.tile([1, 4], f32, tag="sc3")        # s_w, s_wo, s_x
        rowsum = per.tile([128, 96], f32, tag="rowsum")
        inv96 = per.tile([128, 96], f32, tag="inv96")
        s2t = per.tile([1, 2], f32, tag="s2t")
        sct = per.tile([1, 8], f32, tag="sct")

        # ============== phase block 1: P0 + P1 (needs x_hi/x_lo/nw) ==============
        with nc.named_scope("P01_load_qkv"), tc.tile_pool(name="ph01", bufs=1) as ph01:
            x_hi = ph01.tile([128, 6, S], bf16, tag="x_hi")
            x_lo = ph01.tile([128, 6, S], bf16, tag="x_lo")
            nw = ph01.tile([128, 6, 1152], bf16, tag="nw")

            with tc.tile_pool(name="ph0", bufs=1) as ph0, \
                 tc.tile_pool(name="ph0x", bufs=2) as ph0x:
                xm6 = ph0.tile([128, 8], f32)
                for dcc in range(6):
                    xc = ph0x.tile([128, S], f32, tag="xc")
                    nc.sync.dma_start(out=xc[:, :],
                                      in_=xT_d[dcc * 128:(dcc + 1) * 128, :])
                    nc.vector.tensor_copy(x_hi[:, dcc, :], xc[:, :])
                    nc.vector.tensor_tensor(x_lo[:, dcc, :], xc[:, :], x_hi[:, dcc, :],
                                            OP.subtract)
                    nc.vector.tensor_reduce(xm6[:, dcc:dcc + 1], xc[:, :], AX.X, OP.max,
                                            apply_absolute_value=True)
                wqkv = ph0.tile([128, 6, 1152], f32)
                nc.sync.dma_start(out=wqkv[:, :, :], in_=wqkvT_d[:, :].rearrange("(c p) e -> p c e", p=128))
                wo = ph0.tile([128, 6, D], f32)
                nc.sync.dma_start(out=wo[:, :, :], in_=woT_d[:, :].rearrange("(c p) e -> p c e", p=128))

                sc128 = ph0.tile([128, 4], f32)
                nc.vector.tensor_reduce(sc128[:, 0:1], wqkv[:, :, :], AX.XY, OP.max, apply_absolute_value=True)
                nc.vector.tensor_reduce(sc128[:, 1:2], wo[:, :, :], AX.XY, OP.max, apply_absolute_value=True)
                nc.vector.tensor_reduce(sc128[:, 2:3], xm6[:, 0:6], AX.X, OP.max)
                nc.vector.memset(sc128[:, 3:4], 0.0)
                scred = ph0.tile([128, 4], f32)
                nc.gpsimd.partition_all_reduce(scred[:, :], sc128[:, :], channels=128,
                                               reduce_op=bass_isa.ReduceOp.max)
                c1 = allreduce(scred[0:1, :], [1, 4], "c1")

                nc.vector.tensor_scalar(sc3[:, 0:3], c1[:, 0:3], 1e-8, INV_QMAX, OP.max, OP.mult)
                rc3 = ph0.tile([1, 4], f32)
                nc.vector.reciprocal(rc3[:, 0:3], sc3[:, 0:3])
                st1 = ph0.tile([1, 4], f32)
                nc.vector.tensor_scalar_add(st1[:, 0:1], rc3[:, 0:1], 0.0)
                nc.vector.tensor_scalar_add(st1[:, 1:2], rc3[:, 1:2], 0.0)
                nc.vector.tensor_scalar_add(st1[:, 2:3], sc3[:, 2:3], 0.0)
                nc.vector.tensor_scalar_add(st1[:, 3:4], rc3[:, 2:3], 0.0)
                nc.gpsimd.partition_broadcast(bc1[:, :], st1[0:1, :], channels=128)

                nc.vector.tensor_scalar(wqkv[:, :, :], wqkv[:, :, :], bc1[:, 0:1], MAGIC, OP.mult, OP.add)
                nc.vector.tensor_scalar(nw[:, :, :], wqkv[:, :, :], MAGIC, None, OP.subtract)
                nc.vector.tensor_scalar(wo[:, :, :], wo[:, :, :], bc1[:, 1:2], MAGIC, OP.mult, OP.add)
                nc.vector.tensor_scalar(nwo[:, :, :], wo[:, :, :], MAGIC, None, OP.subtract)

            # ---- P1: QKV projection -> stash raw to DRAM, scan per-tensor maxes ----
            with tc.tile_pool(name="ph1", bufs=3) as ph1, \
                 tc.tile_pool(name="ps1", bufs=2, space="PSUM") as ps1:
                qm = per.tile([128, 12], f32, tag="qm")
                for ec in range(9):
                    pt = ps1.tile([128, S], f32, tag="acc")
                    for st in range(4):
                        first = True
                        for dc in range(6):
                            for xp in (x_hi, x_lo):
                                nc.tensor.matmul(pt[:, st * 512:(st + 1) * 512],
                                                 nw[:, dc, ec * 128:(ec + 1) * 128],
                                                 xp[:, dc, st * 512:(st + 1) * 512],
                                                 start=first, stop=(dc == 5 and xp is x_lo))
                                first = False
                    raw = ph1.tile([128, S], f32, tag="raw")
                    nc.scalar.copy(raw[:, :], pt[:, :])
                    nc.vector.tensor_reduce(qm[:, ec:ec + 1], raw[:, :], AX.X, OP.max,
                                            apply_absolute_value=True)
                    nc.sync.dma_start(out=qkv_dram[ec], in_=raw[:, :])
                qm2 = per.tile([128, 4], f32, tag="qm2")
                nc.vector.tensor_reduce(qm2[:, 0:1], qm[:, 0:3], AX.X, OP.max)
                nc.vector.tensor_reduce(qm2[:, 1:2], qm[:, 3:6], AX.X, OP.max)
                nc.vector.tensor_reduce(qm2[:, 2:3], qm[:, 6:9], AX.X, OP.max)
                nc.vector.memset(qm2[:, 3:4], 0.0)
                qm3 = per.tile([128, 4], f32, tag="qm3")
                nc.gpsimd.partition_all_reduce(qm3[:, :], qm2[:, :], channels=128,
                                               reduce_op=bass_isa.ReduceOp.max)
                c2 = allreduce(qm3[0:1, :], [1, 4], "c2")
                nc.vector.tensor_scalar(sc_qkv[:, 0:3], c2[:, 0:3], 1e-8, INV_QMAX, OP.max, OP.mult)
                nc.vector.tensor_scalar(sc_qkv[:, 3:4], sc_qkv[:, 0:1], sc_qkv[:, 1:2], 0.125,
                                        OP.mult, OP.mult)
                rq = per.tile([1, 4], f32, tag="rq")
                nc.vector.reciprocal(rq[:, 0:3], sc_qkv[:, 0:3])
                nc.gpsimd.partition_broadcast(bc2[:, 0:3], rq[0:1, 0:3], channels=128)

        # ============== phase block 2: nv (alive until pass C end) ==============
        with tc.tile_pool(name="phV", bufs=1) as phV:
            nv = [phV.tile([128, 16, 128], bf16, tag=f"nv{i}", name=f"nv{i}") for i in range(3)]
            ctxf = [phV.tile([128, S], f32, tag=f"ctxf{i}", name=f"ctxf{i}") for i in range(3)]

            # -------- phase block 3: nqk (alive through pass A+B) --------
            with nc.named_scope("PAB"), tc.tile_pool(name="phAB", bufs=1) as phAB:
                nqk = phAB.tile([128, 6, S], bf16, tag="nqk")
                with nc.named_scope("P1b_quant"), tc.tile_pool(name="ph1b", bufs=3) as ph1b:
                    for ec in range(9):
                        raw = ph1b.tile([128, S], f32, tag="raw2")
                        nc.sync.dma_start(out=raw[:, :], in_=qkv_dram[ec])
                        t1 = ph1b.tile([128, S], f32, tag="t1")
                        ci = 0 if ec < 3 else (1 if ec < 6 else 2)
                        nc.vector.tensor_scalar(t1[:, :], raw[:, :], bc2[:, ci:ci + 1], MAGIC,
                                                OP.mult, OP.add)
                        if ec < 6:
                            nc.vector.tensor_scalar(nqk[:, ec, :], t1[:, :], MAGIC, None, OP.subtract)
                        else:
                            nvt = ph1b.tile([128, S], bf16, tag="nvt")
                            nc.vector.tensor_scalar(nvt[:, :], t1[:, :], MAGIC, None, OP.subtract)
                            nc.sync.dma_start_transpose(nv[ec - 6][:, :, :], nvt[:, :])

                # -------- P2 (pass A): global |scores| max --------
                am = per.tile([128, 200], f32, tag="am")
                with nc.named_scope("P2_passA"), tc.tile_pool(name="ps2", bufs=2, space="PSUM") as ps2:
                    idx = 0
                    for hp in range(3):
                        for qt in range(16):
                            for kh in range(2):
                                pa = ps2.tile([128, 1024], f32, tag="pa")
                                pb = ps2.tile([128, 1024], f32, tag="pb")
                                for k2 in range(2):
                                    kk = kh * 2 + k2
                                    nc.tensor.matmul(pa[:, k2 * 512:(k2 + 1) * 512],
                                                     nqk[0:64, hp, qt * 128:(qt + 1) * 128],
                                                     nqk[0:64, 3 + hp, kk * 512:(kk + 1) * 512],
                                                     start=True, stop=True,
                                                     tile_position=(0, 0), skip_group_check=True)
                                    nc.tensor.matmul(pb[:, k2 * 512:(k2 + 1) * 512],
                                                     nqk[64:128, hp, qt * 128:(qt + 1) * 128],
                                                     nqk[64:128, 3 + hp, kk * 512:(kk + 1) * 512],
                                                     start=True, stop=True,
                                                     tile_position=(64, 0), skip_group_check=True)
                                if idx < 198:
                                    nc.vector.tensor_reduce(am[:, idx:idx + 1], pa[:, :], AX.X,
                                                            OP.max, apply_absolute_value=True)
                                    nc.vector.tensor_reduce(am[:, idx + 1:idx + 2], pb[:, :], AX.X,
                                                            OP.max, apply_absolute_value=True)
                                idx += 2
                    amr = per.tile([128, 1], f32, tag="amr")
                    nc.vector.tensor_reduce(amr[:, 0:1], am[:, 0:192], AX.X, OP.max)
                    amr2 = per.tile([128, 1], f32, tag="amr2")
                    nc.gpsimd.partition_all_reduce(amr2[:, :], amr[:, :], channels=128,
                                                   reduce_op=bass_isa.ReduceOp.max)
                    c3 = allreduce(amr2[0:1, :], [1, 1], "c3")
                    t = per.tile([1, 4], f32, tag="s1t")
                    nc.vector.tensor_scalar(t[:, 0:1], c3[:, 0:1], sc_qkv[:, 3:4], None, OP.mult)
                    nc.vector.tensor_scalar(t[:, 1:2], t[:, 0:1], 1e-8, INV_QMAX, OP.max, OP.mult)
                    nc.vector.reciprocal(t[:, 2:3], t[:, 1:2])
                    nc.vector.tensor_scalar(t[:, 3:4], t[:, 2:3], sc_qkv[:, 3:4], None, OP.mult)
                    nc.gpsimd.partition_broadcast(bc2[:, 3:4], t[0:1, 1:2], channels=128)  # s1
                    nc.gpsimd.partition_broadcast(bc2[:, 4:5], t[0:1, 3:4], channels=128)  # c

                # -------- P3 (pass B): n_s int8 + rowmax + rowsum --------
                rm = per.tile([128, 192], f32, tag="rm")
                rs = per.tile([128, 192], f32, tag="rs")
                with nc.named_scope("P3_passB"), tc.tile_pool(name="ph3", bufs=4) as ph3, \
                     tc.tile_pool(name="ps3", bufs=2, space="PSUM") as ps3:
                    for hp in range(3):
                        for qt in range(16):
                            for kh in range(2):
                                pa = ps3.tile([128, 1024], f32, tag="pa")
                                pb = ps3.tile([128, 1024], f32, tag="pb")
                                for k2 in range(2):
                                    kk = kh * 2 + k2
                                    nc.tensor.matmul(pa[:, k2 * 512:(k2 + 1) * 512],
                                                     nqk[0:64, hp, qt * 128:(qt + 1) * 128],
                                                     nqk[0:64, 3 + hp, kk * 512:(kk + 1) * 512],
                                                     start=True, stop=True,
                                                     tile_position=(0, 0), skip_group_check=True)
                                    nc.tensor.matmul(pb[:, k2 * 512:(k2 + 1) * 512],
                                                     nqk[64:128, hp, qt * 128:(qt + 1) * 128],
                                                     nqk[64:128, 3 + hp, kk * 512:(kk + 1) * 512],
                                                     start=True, stop=True,
                                                     tile_position=(64, 0), skip_group_check=True)
                                for hd, pt_ in ((0, pa), (1, pb)):
                                    hloc = 2 * hp + hd
                                    slot = kh * 96 + hloc * 16 + qt
                                    ns_t = ph3.tile([128, 1024], i8, tag="ns")
                                    nc.vector.tensor_scalar(ns_t[:, :], pt_[:, :], bc2[:, 4:5],
                                                            None, OP.mult, OP.max,
                                                            accum_out=rm[:, slot:slot + 1])
                                    pe_t = ph3.tile([128, 1024], f32, tag="pe")
                                    nc.scalar.activation(pe_t[:, :], ns_t[:, :], AF.Exp,
                                                         bias=0.0, scale=bc2[:, 3:4],
                                                         accum_out=rs[:, slot:slot + 1])
                                    nc.sync.dma_start(out=ns_dram[hp, hd, qt, kh], in_=ns_t[:, :])

                # combine halves, m2, s2, inv per row
                with tc.tile_pool(name="ph3b", bufs=1) as ph3b:
                    nc.vector.tensor_tensor(rowsum[:, :], rs[:, 0:96], rs[:, 96:192], OP.add)
                    rmax = ph3b.tile([128, 96], f32)
                    nc.vector.tensor_tensor(rmax[:, :], rm[:, 0:96], rm[:, 96:192], OP.max)
                    nrm = ph3b.tile([128, 96], i8)
                    nc.vector.tensor_scalar(nrm[:, :], rmax[:, :], 0.0, None, OP.add)
                    erm = ph3b.tile([128, 96], f32)
                    nc.scalar.activation(erm[:, :], nrm[:, :], AF.Exp, bias=0.0, scale=bc2[:, 3:4])
                    rrs = ph3b.tile([128, 96], f32)
                    nc.vector.reciprocal(rrs[:, :], rowsum[:, :])
                    amax = ph3b.tile([128, 96], f32)
                    nc.vector.tensor_tensor(amax[:, :], erm[:, :], rrs[:, :], OP.mult)
                    am1 = ph3b.tile([128, 1], f32)
                    nc.vector.tensor_reduce(am1[:, 0:1], amax[:, :], AX.X, OP.max)
                    am2 = ph3b.tile([128, 1], f32)
                    nc.gpsimd.partition_all_reduce(am2[:, :], am1[:, :], channels=128,
                                                   reduce_op=bass_isa.ReduceOp.max)
                    c4 = allreduce(am2[0:1, :], [1, 1], "c4")
                    nc.vector.tensor_scalar(s2t[:, 0:1], c4[:, 0:1], 1e-8, INV_QMAX, OP.max, OP.mult)
                    s2b = per.tile([128, 1], f32, tag="s2b")
                    nc.gpsimd.partition_broadcast(s2b[:, :], s2t[0:1, 0:1], channels=128)
                    t96 = ph3b.tile([128, 96], f32)
                    nc.vector.tensor_scalar(t96[:, :], rowsum[:, :], s2b[:, 0:1], None, OP.mult)
                    nc.vector.reciprocal(inv96[:, :], t96[:, :])

            # -------- P4 (pass C): attn quant + transpose + ctx --------
            cm = per.tile([128, 4], f32, tag="cm")
            with nc.named_scope("P4_passC"), tc.tile_pool(name="ph4", bufs=4) as ph4, \
                 tc.tile_pool(name="ph4t", bufs=6) as ph4t, \
                 tc.tile_pool(name="ps4", bufs=2, space="PSUM") as ps4:
                for hp in range(3):
                    cps = ps4.tile([128, S], f32, tag="ctx")
                    for hd in range(2):
                        hloc = 2 * hp + hd
                        for qg in range(4):
                            nats = []
                            for qi in range(4):
                                qt = qg * 4 + qi
                                ns_t = ph4.tile([128, 2, 1024], i8, tag="nsr", bufs=3)
                                nc.sync.dma_start(out=ns_t[:, :, :],
                                                  in_=ns_dram[hp, hd, qt].rearrange("a p f -> p a f"))
                                pe_t = ph4.tile([128, S], f32, tag="pe4", bufs=3)
                                nc.scalar.activation(pe_t[:, :],
                                                     ns_t[:, :, :].rearrange("p a f -> p (a f)"),
                                                     AF.Exp, bias=0.0, scale=bc2[:, 3:4])
                                na8 = ph4.tile([128, S], i8, tag="na8", bufs=3)
                                slot = hloc * 16 + qt
                                nc.vector.tensor_scalar(na8[:, :], pe_t[:, :],
                                                        inv96[:, slot:slot + 1], None, OP.mult)
                                nab = ph4.tile([128, S], bf16, tag="nab", bufs=4)
                                if qt % 2 == 0:
                                    nc.scalar.copy(nab[:, :], na8[:, :])
                                else:
                                    nc.vector.tensor_copy(nab[:, :], na8[:, :])
                                nat = ph4t.tile([128, 16, 128], bf16, tag="nat", bufs=10)
                                nc.sync.dma_start_transpose(nat[:, :, :], nab[:, :])
                                nats.append((qt, nat))
                            for kc in range(16):
                                for qt, nat in nats:
                                    nc.tensor.matmul(
                                        cps[hd * 64:(hd + 1) * 64, qt * 128:(qt + 1) * 128],
                                        nv[hp][:, kc, hd * 64:(hd + 1) * 64],
                                        nat[:, kc, :],
                                        start=(kc == 0), stop=(kc == 15),
                                        tile_position=(0, hd * 64), skip_group_check=True)
                    nc.vector.tensor_reduce(cm[:, hp:hp + 1], cps[:, :], AX.X, OP.max,
                                            apply_absolute_value=True)
                    nc.scalar.copy(ctxf[hp][:, :], cps[:, :])
                cmr = per.tile([128, 1], f32, tag="cmr")
                nc.vector.tensor_reduce(cmr[:, 0:1], cm[:, 0:3], AX.X, OP.max)
                cmr2 = per.tile([128, 1], f32, tag="cmr2")
                nc.gpsimd.partition_all_reduce(cmr2[:, :], cmr[:, :], channels=128,
                                               reduce_op=bass_isa.ReduceOp.max)
                c5 = allreduce(cmr2[0:1, :], [1, 1], "c5")

            # quantize ctx -> int8 and ship via pair AllGather
            with nc.named_scope("P5_exchange"), tc.tile_pool(name="ph5", bufs=2) as ph5, \
                 tc.tile_pool(name="dram5", bufs=1, space="DRAM") as dram5:
                nc.vector.tensor_scalar(sct[:, 0:1], s2t[:, 0:1], sc_qkv[:, 2:3], None, OP.mult)
                nc.vector.tensor_scalar(sct[:, 1:2], c5[:, 0:1], sct[:, 0:1], None, OP.mult)
                nc.vector.tensor_scalar(sct[:, 2:3], sct[:, 1:2], 1e-8, INV_QMAX, OP.max, OP.mult)
                rcc = per.tile([1, 2], f32, tag="rcc")
                nc.vector.reciprocal(rcc[:, 0:1], sct[:, 2:3])
                nc.vector.tensor_scalar(sct[:, 3:4], rcc[:, 0:1], sct[:, 0:1], None, OP.mult)  # c2
                nc.vector.tensor_scalar(sct[:, 4:5], sct[:, 2:3], sc3[:, 1:2], None, OP.mult)  # gy
                bc5 = per.tile([128, 1], f32, tag="bc5")
                nc.gpsimd.partition_broadcast(bc5[:, :], sct[0:1, 3:4], channels=128)
                gin = dram5.tile([3, 128, S], i8)
                for hp in range(3):
                    q8 = ph5.tile([128, S], i8, tag="q8")
                    nc.vector.tensor_scalar(q8[:, :], ctxf[hp][:, :], bc5[:, 0:1], None, OP.mult)
                    nc.gpsimd.dma_start(gin[hp], q8[:, :])
                gout = dram5.tile([2, 3, 128, S], i8)
                nc.gpsimd.collective_compute("AllGather", OP.bypass, replica_groups=RG_PAIR,
                                             ins=[gin.opt()], outs=[gout.opt()])
                nctx = phV.tile([128, 6, S], bf16, tag="nctx")
                for half in range(2):
                    for hp in range(3):
                        t8 = ph5.tile([128, S], i8, tag="t8")
                        nc.sync.dma_start(out=t8[:, :], in_=gout[half, hp])
                        nc.vector.tensor_copy(nctx[:, half * 3 + hp, :], t8[:, :])

            # ============== P5: output projection + residual + BN ==============
            with nc.named_scope("P6_out"), tc.tile_pool(name="ph6", bufs=2) as ph6, \
                 tc.tile_pool(name="ph6r", bufs=1) as ph6r, \
                 tc.tile_pool(name="ps6", bufs=2, space="PSUM") as ps6, \
                 tc.tile_pool(name="ps6b", bufs=1, space="PSUM") as ps6b:
                ones = ph6r.tile([128, 1], f32)
                nc.vector.memset(ones[:, :], 1.0)
                x8a = ph6r.tile([128, 16, D], i8)
                for st in range(16):
                    xt_ = ph6.tile([128, D], f32, tag="xt")
                    nc.sync.dma_start(out=xt_[:, :], in_=xr_d[:, st, :])
                    nc.vector.tensor_scalar(x8a[:, st, :], xt_[:, :], bc1[:, 3:4], None, OP.mult)
                ym = per.tile([128, 16], f32, tag="ym")
                rtile = ph6r.tile([128, 16, D], f32)
                sumr = ps6b.tile([1, 1024], f32, tag="sr1")
                sumr2 = ps6b.tile([1, 1024], f32, tag="sr2")
                for st in range(16):
                    yp = ps6.tile([128, D], f32, tag="yp")
                    for dc in range(6):
                        nc.tensor.matmul(yp[:, 0:512], nctx[:, dc, st * 128:(st + 1) * 128],
                                         nwo[:, dc, 0:512], start=(dc == 0), stop=(dc == 5))
                        nc.tensor.matmul(yp[:, 512:768], nctx[:, dc, st * 128:(st + 1) * 128],
                                         nwo[:, dc, 512:768], start=(dc == 0), stop=(dc == 5))
                    nc.vector.tensor_reduce(ym[:, st:st + 1], yp[:, :], AX.X, OP.max,
                                            apply_absolute_value=True)
                    nc.scalar.copy(rtile[:, st, :], yp[:, :])
                ymr = per.tile([128, 1], f32, tag="ymr")
                nc.vector.tensor_reduce(ymr[:, 0:1], ym[:, :], AX.X, OP.max)
                ymr2 = per.tile([128, 1], f32, tag="ymr2")
                nc.gpsimd.partition_all_reduce(ymr2[:, :], ymr[:, :], channels=128,
                                               reduce_op=bass_isa.ReduceOp.max)
                c6 = allreduce(ymr2[0:1, :], [1, 1], "c6")
                syt = per.tile([1, 4], f32, tag="syt")
                nc.vector.tensor_scalar(syt[:, 0:1], c6[:, 0:1], sct[:, 4:5], None, OP.mult)
                nc.vector.tensor_scalar(syt[:, 1:2], syt[:, 0:1], 1e-8, INV_QMAX, OP.max, OP.mult)
                nc.vector.reciprocal(syt[:, 2:3], syt[:, 1:2])
                nc.vector.tensor_scalar(syt[:, 3:4], syt[:, 2:3], sct[:, 4:5], None, OP.mult)
                bc6 = per.tile([128, 2], f32, tag="bc6")
                nc.gpsimd.partition_broadcast(bc6[:, 0:1], syt[0:1, 3:4], channels=128)  # cgy
                nc.gpsimd.partition_broadcast(bc6[:, 1:2], syt[0:1, 1:2], channels=128)  # s_y
                for st in range(16):
                    fqx = ph6.tile([128, D], f32, tag="fqx")
                    nc.vector.tensor_scalar(fqx[:, :], x8a[:, st, :], bc1[:, 2:3], None, OP.mult)
                    y8 = ph6.tile([128, D], i8, tag="y8")
                    nc.vector.tensor_scalar(y8[:, :], rtile[:, st, :], bc6[:, 0:1], None, OP.mult)
                    fqy = ph6.tile([128, D], f32, tag="fqy", bufs=1)
                    nc.vector.tensor_scalar(fqy[:, :], y8[:, :], bc6[:, 1:2], None, OP.mult)
                    nc.vector.tensor_tensor(rtile[:, st, :], fqy[:, :], fqx[:, :], OP.add)
                    nc.tensor.matmul(sumr[:, 0:512], ones[:, :], rtile[:, st, 0:512],
                                     start=(st == 0), stop=(st == 15))
                    nc.tensor.matmul(sumr[:, 512:768], ones[:, :], rtile[:, st, 512:768],
                                     start=(st == 0), stop=(st == 15))
                    r2 = ph6.tile([128, D], f32, tag="r2", bufs=1)
                    nc.scalar.square(r2[:, :], rtile[:, st, :])
                    nc.tensor.matmul(sumr2[:, 0:512], ones[:, :], r2[:, 0:512],
                                     start=(st == 0), stop=(st == 15))
                    nc.tensor.matmul(sumr2[:, 512:768], ones[:, :], r2[:, 512:768],
                                     start=(st == 0), stop=(st == 15))
                sums = ph6r.tile([1, 2 * D], f32)
                nc.vector.tensor_copy(sums[:, 0:768], sumr[:, 0:768])
                nc.vector.tensor_copy(sums[:, 768:1536], sumr2[:, 0:768])
                c7 = allreduce(sums[:, :], [1, 2 * D], "c7", op=OP.add)
                mean = ph6r.tile([1, D], f32)
                nc.vector.tensor_scalar(mean[:, :], c7[:, 0:768], N_INV, None, OP.mult)
                msq = ph6r.tile([1, D], f32)
                nc.vector.tensor_tensor(msq[:, :], mean[:, :], mean[:, :], OP.mult)
                var0 = ph6r.tile([1, D], f32)
                nc.vector.tensor_scalar(var0[:, :], c7[:, 768:1536], N_INV, None, OP.mult)
                var = ph6r.tile([1, D], f32)
                nc.vector.tensor_tensor(var[:, :], var0[:, :], msq[:, :], OP.subtract)
                vare = ph6r.tile([1, D], f32)
                nc.vector.tensor_scalar(vare[:, :], var[:, :], BN_EPS, None, OP.add)
                sd = ph6r.tile([1, D], f32)
                nc.scalar.activation(sd[:, :], vare[:, :], AF.Sqrt, bias=0.0, scale=1.0)
                invstd = ph6r.tile([1, D], f32)
                nc.vector.reciprocal(invstd[:, :], sd[:, :])
                mm = ph6r.tile([1, D], f32)
                nc.vector.tensor_tensor(mm[:, :], mean[:, :], invstd[:, :], OP.mult)
                invstdB = ph6r.tile([128, D], f32)
                nc.gpsimd.partition_broadcast(invstdB[:, :], invstd[0:1, :], channels=128)
                mmB = ph6r.tile([128, D], f32)
                nc.gpsimd.partition_broadcast(mmB[:, :], mm[0:1, :], channels=128)
                rnm = per.tile([128, 16], f32, tag="rnm")
                for st in range(16):
                    t_ = ph6.tile([128, D], f32, tag="rn_t")
                    nc.vector.tensor_tensor(t_[:, :], rtile[:, st, :], invstdB[:, :], OP.mult)
                    nc.vector.tensor_tensor(rtile[:, st, :], t_[:, :], mmB[:, :], OP.subtract)
                    nc.vector.tensor_reduce(rnm[:, st:st + 1], rtile[:, st, :], AX.X, OP.max,
                                            apply_absolute_value=True)
                rnr = per.tile([128, 1], f32, tag="rnr")
                nc.vector.tensor_reduce(rnr[:, 0:1], rnm[:, :], AX.X, OP.max)
                rnr2 = per.tile([128, 1], f32, tag="rnr2")
                nc.gpsimd.partition_all_reduce(rnr2[:, :], rnr[:, :], channels=128,
                                               reduce_op=bass_isa.ReduceOp.max)
                c8 = allreduce(rnr2[0:1, :], [1, 1], "c8")
                srt = per.tile([1, 2], f32, tag="srt")
                nc.vector.tensor_scalar(srt[:, 0:1], c8[:, 0:1], 1e-8, INV_QMAX, OP.max, OP.mult)
                nc.vector.reciprocal(srt[:, 1:2], srt[:, 0:1])
                bc8 = per.tile([128, 2], f32, tag="bc8")
                nc.gpsimd.partition_broadcast(bc8[:, 0:1], srt[0:1, 1:2], channels=128)
                nc.gpsimd.partition_broadcast(bc8[:, 1:2], srt[0:1, 0:1], channels=128)
                for st in range(16):
                    o8 = ph6.tile([128, D], i8, tag="o8")
                    nc.vector.tensor_scalar(o8[:, :], rtile[:, st, :], bc8[:, 0:1], None, OP.mult)
                    of = ph6.tile([128, D], f32, tag="of", bufs=1)
                    nc.vector.tensor_scalar(of[:, :], o8[:, :], bc8[:, 1:2], None, OP.mult)
                    nc.sync.dma_start(out=out_d[:, st, :], in_=of[:, :])

    nc.finalize()
    return nc


def _prep_inputs(x, w_in, w_out):
    """Host-side sharding (data movement only)."""
    ins = []
    woT = np.ascontiguousarray(w_out.T)
    for c in range(NCORES):
        b, hs = c // 2, c % 2
        heads = list(range(6 * hs, 6 * hs + 6))
        rows = []
        for base in (0, D, 2 * D):
            for h in heads:
                rows.append(w_in[base + h * DH: base + (h + 1) * DH])
        w_sel = np.concatenate(rows, axis=0)            # [1152, 768]
        ins.append({
            "xT": np.ascontiguousarray(x[b].T),
            "xr": np.ascontiguousarray(x[b].reshape(16, 128, D).transpose(1, 0, 2)),
            "wqkvT": np.ascontiguousarray(w_sel.T),
            "woT": woT,
        })
    return ins


def kernel(x, w_in, w_out):
    from concourse import bass2jax
    if "nc" not in _CACHE:
        _CACHE["nc"] = _build()
    nc = _CACHE["nc"]
    ins = _prep_inputs(np.asarray(x, np.float32), np.asarray(w_in, np.float32),
                       np.asarray(w_out, np.float32))
    res = bass2jax.run_bass_via_pjrt(nc, ins, n_cores=NCORES)
    out = np.empty((B, S, D), np.float32)
    for c in range(NCORES):
        b, hs = c // 2, c % 2
        full = res[c]["out"].transpose(1, 0, 2).reshape(S, D)
        out[b, hs * 1024:(hs + 1) * 1024] = full[hs * 1024:(hs + 1) * 1024]
    return out


# revision 17
# speedup vs baseline: 1.0248x; 1.0248x over previous
"""Trainium2 Bass kernel for a quantized multi-head attention block (8-core SPMD).

Sharding: core c handles batch b=c//2 and global heads [6*(c%2), 6*(c%2)+6).
Attention is head-parallel; the output projection + residual + batchnorm run
pair-redundantly over the full sequence of the core's batch (the host keeps
the rows it needs). Cross-core reductions (quant scales, BN stats) use
AllReduce; the head->channel re-shard uses a pair-local AllGather of
int8-quantized context.
"""
import sys
sys.path.insert(0, "/opt/trn_rl_repo")
import numpy as np

B, S, D = 4, 2048, 768
H, DH, HLOC, NCORES = 12, 64, 6, 8
INV_QMAX = float(np.float32(1.0) / np.float32(127.0))
MAGIC = 12582912.0  # 1.5 * 2^23: fp32 add forces round-to-nearest-even
BN_EPS = 1e-5
N_INV = float(np.float32(1.0) / np.float32(2 * B * S))  # pair-redundant BN count

_CACHE = {}


def _build():
    import concourse.bacc as bacc
    import concourse.tile as tile
    from concourse import mybir, bass_isa

    f32, bf16, i8 = mybir.dt.float32, mybir.dt.bfloat16, mybir.dt.int8
    AF, OP, AX = mybir.ActivationFunctionType, mybir.AluOpType, mybir.AxisListType

    nc = bacc.Bacc()

    xT_d = nc.dram_tensor("xT", [D, S], f32, kind="ExternalInput")            # x[b].T
    xr_d = nc.dram_tensor("xr", [128, 16, D], f32, kind="ExternalInput")      # x[b] rows tiled
    wqkvT_d = nc.dram_tensor("wqkvT", [D, 1152], f32, kind="ExternalInput")   # w slice, transposed
    woT_d = nc.dram_tensor("woT", [D, D], f32, kind="ExternalInput")          # w_out.T
    out_d = nc.dram_tensor("out", [128, 16, D], f32, kind="ExternalOutput")

    qkv_dram = nc.dram_tensor("qkv_stash", [9, 128, S], f32)
    ns_dram = nc.dram_tensor("ns_stash", [3, 2, 16, 2, 128, 1024], i8)
    RG_ALL = [list(range(NCORES))]
    RG_PAIR = [[0, 1], [2, 3], [4, 5], [6, 7]]

    with tile.TileContext(nc) as tc, \
         tc.tile_pool(name="per", bufs=1) as per, \
         tc.tile_pool(name="drampool", bufs=1, space="DRAM") as dram:

        def allreduce(src_ap, shp, tag, op=OP.max):
            bi = dram.tile(shp, f32, tag=f"ari_{tag}")
            bo = dram.tile(shp, f32, tag=f"aro_{tag}")
            nc.gpsimd.dma_start(bi[:], src_ap)
            nc.gpsimd.collective_compute("AllReduce", op, replica_groups=RG_ALL,
                                         ins=[bi.opt()], outs=[bo.opt()])
            r = per.tile(shp, f32, tag=f"arr_{tag}")
            nc.gpsimd.dma_start(r[:], bo[:])
            return r

        # persistent small tiles
        nwo = per.tile([128, 6, D], bf16, tag="nwo")
        bc1 = per.tile([128, 4], f32, tag="bc1")      # cw, cwo, s_x, 1/s_x
        bc2 = per.tile([128, 8], f32, tag="bc2")      # cq, ck, cv, s1, c
        sc_qkv = per.tile([1, 4], f32, tag="sc_qkv")  # s_q, s_k, s_v, g
        sc3 = per.tile([1, 4], f32, tag="sc3")        # s_w, s_wo, s_x
        rowsum = per.tile([128, 96], f32, tag="rowsum")
        inv96 = per.tile([128, 96], f32, tag="inv96")
        s2t = per.tile([1, 2], f32, tag="s2t")
        sct = per.tile([1, 8], f32, tag="sct")

        # ============== phase block 1: P0 + P1 (needs x_hi/x_lo/nw) ==============
        with nc.named_scope("P01_load_qkv"), tc.tile_pool(name="ph01", bufs=1) as ph01:
            x_hi = ph01.tile([128, 6, S], bf16, tag="x_hi")
            x_lo = ph01.tile([128, 6, S], bf16, tag="x_lo")
            nw = ph01.tile([128, 6, 1152], bf16, tag="nw")

            with tc.tile_pool(name="ph0", bufs=1) as ph0, \
                 tc.tile_pool(name="ph0x", bufs=2) as ph0x:
                xm6 = ph0.tile([128, 8], f32)
                for dcc in range(6):
                    xc = ph0x.tile([128, S], f32, tag="xc")
                    nc.sync.dma_start(out=xc[:, :],
                                      in_=xT_d[dcc * 128:(dcc + 1) * 128, :])
                    nc.vector.tensor_copy(x_hi[:, dcc, :], xc[:, :])
                    nc.vector.tensor_tensor(x_lo[:, dcc, :], xc[:, :], x_hi[:, dcc, :],
                                            OP.subtract)
                    nc.vector.tensor_reduce(xm6[:, dcc:dcc + 1], xc[:, :], AX.X, OP.max,
                                            apply_absolute_value=True)
                wqkv = ph0.tile([128, 6, 1152], f32)
                nc.sync.dma_start(out=wqkv[:, :, :], in_=wqkvT_d[:, :].rearrange("(c p) e -> p c e", p=128))
                wo = ph0.tile([128, 6, D], f32)
                nc.sync.dma_start(out=wo[:, :, :], in_=woT_d[:, :].rearrange("(c p) e -> p c e", p=128))

                sc128 = ph0.tile([128, 4], f32)
                nc.vector.tensor_reduce(sc128[:, 0:1], wqkv[:, :, :], AX.XY, OP.max, apply_absolute_value=True)
                nc.vector.tensor_reduce(sc128[:, 1:2], wo[:, :, :], AX.XY, OP.max, apply_absolute_value=True)
                nc.vector.tensor_reduce(sc128[:, 2:3], xm6[:, 0:6], AX.X, OP.max)
                nc.vector.memset(sc128[:, 3:4], 0.0)
                scred = ph0.tile([128, 4], f32)
                nc.gpsimd.partition_all_reduce(scred[:, :], sc128[:, :], channels=128,
                                               reduce_op=bass_isa.ReduceOp.max)
                c1 = allreduce(scred[0:1, :], [1, 4], "c1")

                nc.vector.tensor_scalar(sc3[:, 0:3], c1[:, 0:3], 1e-8, INV_QMAX, OP.max, OP.mult)
                rc3 = ph0.tile([1, 4], f32)
                nc.vector.reciprocal(rc3[:, 0:3], sc3[:, 0:3])
                st1 = ph0.tile([1, 4], f32)
                nc.vector.tensor_scalar_add(st1[:, 0:1], rc3[:, 0:1], 0.0)
                nc.vector.tensor_scalar_add(st1[:, 1:2], rc3[:, 1:2], 0.0)
                nc.vector.tensor_scalar_add(st1[:, 2:3], sc3[:, 2:3], 0.0)
                nc.vector.tensor_scalar_add(st1[:, 3:4], rc3[:, 2:3], 0.0)
                nc.gpsimd.partition_broadcast(bc1[:, :], st1[0:1, :], channels=128)

                nc.vector.tensor_scalar(wqkv[:, :, :], wqkv[:, :, :], bc1[:, 0:1], MAGIC, OP.mult, OP.add)
                nc.vector.tensor_scalar(nw[:, :, :], wqkv[:, :, :], MAGIC, None, OP.subtract)
                nc.vector.tensor_scalar(wo[:, :, :], wo[:, :, :], bc1[:, 1:2], MAGIC, OP.mult, OP.add)
                nc.vector.tensor_scalar(nwo[:, :, :], wo[:, :, :], MAGIC, None, OP.subtract)

            # ---- P1: QKV projection -> stash raw to DRAM, scan per-tensor maxes ----
            with tc.tile_pool(name="ph1", bufs=3) as ph1, \
                 tc.tile_pool(name="ps1", bufs=2, space="PSUM") as ps1:
                qm = per.tile([128, 12], f32, tag="qm")
                for ec in range(9):
                    pt = ps1.tile([128, S], f32, tag="acc")
                    for st in range(4):
                        first = True
                        for dc in range(6):
                            for xp in (x_hi, x_lo):
                                nc.tensor.matmul(pt[:, st * 512:(st + 1) * 512],
                                                 nw[:, dc, ec * 128:(ec + 1) * 128],
                                                 xp[:, dc, st * 512:(st + 1) * 512],
                                                 start=first, stop=(dc == 5 and xp is x_lo))
                                first = False
                    raw = ph1.tile([128, S], f32, tag="raw")
                    nc.scalar.copy(raw[:, :], pt[:, :])
                    nc.vector.tensor_reduce(qm[:, ec:ec + 1], raw[:, :], AX.X, OP.max,
                                            apply_absolute_value=True)
                    nc.sync.dma_start(out=qkv_dram[ec], in_=raw[:, :])
                qm2 = per.tile([128, 4], f32, tag="qm2")
                nc.vector.tensor_reduce(qm2[:, 0:1], qm[:, 0:3], AX.X, OP.max)
                nc.vector.tensor_reduce(qm2[:, 1:2], qm[:, 3:6], AX.X, OP.max)
                nc.vector.tensor_reduce(qm2[:, 2:3], qm[:, 6:9], AX.X, OP.max)
                nc.vector.memset(qm2[:, 3:4], 0.0)
                qm3 = per.tile([128, 4], f32, tag="qm3")
                nc.gpsimd.partition_all_reduce(qm3[:, :], qm2[:, :], channels=128,
                                               reduce_op=bass_isa.ReduceOp.max)
                c2 = allreduce(qm3[0:1, :], [1, 4], "c2")
                nc.vector.tensor_scalar(sc_qkv[:, 0:3], c2[:, 0:3], 1e-8, INV_QMAX, OP.max, OP.mult)
                nc.vector.tensor_scalar(sc_qkv[:, 3:4], sc_qkv[:, 0:1], sc_qkv[:, 1:2], 0.125,
                                        OP.mult, OP.mult)
                rq = per.tile([1, 4], f32, tag="rq")
                nc.vector.reciprocal(rq[:, 0:3], sc_qkv[:, 0:3])
                nc.gpsimd.partition_broadcast(bc2[:, 0:3], rq[0:1, 0:3], channels=128)

        # ============== phase block 2: nv (alive until pass C end) ==============
        phO_cm = tc.tile_pool(name="phO", bufs=1)
        phO = phO_cm.__enter__()
        phV_cm = tc.tile_pool(name="phV", bufs=1)
        phV = phV_cm.__enter__()
        if True:
            nv = [phV.tile([128, 16, 128], bf16, tag=f"nv{i}", name=f"nv{i}") for i in range(3)]
            ctxf = [phV.tile([128, S], f32, tag=f"ctxf{i}", name=f"ctxf{i}") for i in range(3)]

            # -------- phase block 3: nqk (alive through pass A+B) --------
            with nc.named_scope("PAB"), tc.tile_pool(name="phAB", bufs=1) as phAB:
                nqk = phAB.tile([128, 6, S], bf16, tag="nqk")
                with nc.named_scope("P1b_quant"), tc.tile_pool(name="ph1b", bufs=3) as ph1b:
                    for ec in range(9):
                        raw = ph1b.tile([128, S], f32, tag="raw2")
                        nc.sync.dma_start(out=raw[:, :], in_=qkv_dram[ec])
                        t1 = ph1b.tile([128, S], f32, tag="t1")
                        ci = 0 if ec < 3 else (1 if ec < 6 else 2)
                        nc.vector.tensor_scalar(t1[:, :], raw[:, :], bc2[:, ci:ci + 1], MAGIC,
                                                OP.mult, OP.add)
                        if ec < 6:
                            nc.vector.tensor_scalar(nqk[:, ec, :], t1[:, :], MAGIC, None, OP.subtract)
                        else:
                            nvt = ph1b.tile([128, S], bf16, tag="nvt")
                            nc.vector.tensor_scalar(nvt[:, :], t1[:, :], MAGIC, None, OP.subtract)
                            nc.sync.dma_start_transpose(nv[ec - 6][:, :, :], nvt[:, :])

                # -------- P2 (pass A): global |scores| max --------
                am = per.tile([128, 200], f32, tag="am")
                with nc.named_scope("P2_passA"), tc.tile_pool(name="ps2", bufs=2, space="PSUM") as ps2:
                    idx = 0
                    for hp in range(3):
                        for qt in range(16):
                            for kh in range(2):
                                pa = ps2.tile([128, 1024], f32, tag="pa")
                                pb = ps2.tile([128, 1024], f32, tag="pb")
                                for k2 in range(2):
                                    kk = kh * 2 + k2
                                    nc.tensor.matmul(pa[:, k2 * 512:(k2 + 1) * 512],
                                                     nqk[0:64, hp, qt * 128:(qt + 1) * 128],
                                                     nqk[0:64, 3 + hp, kk * 512:(kk + 1) * 512],
                                                     start=True, stop=True,
                                                     tile_position=(0, 0), skip_group_check=True)
                                    nc.tensor.matmul(pb[:, k2 * 512:(k2 + 1) * 512],
                                                     nqk[64:128, hp, qt * 128:(qt + 1) * 128],
                                                     nqk[64:128, 3 + hp, kk * 512:(kk + 1) * 512],
                                                     start=True, stop=True,
                                                     tile_position=(64, 0), skip_group_check=True)
                                if idx < 198:
                                    nc.vector.tensor_reduce(am[:, idx:idx + 1], pa[:, :], AX.X,
                                                            OP.max, apply_absolute_value=True)
                                    nc.vector.tensor_reduce(am[:, idx + 1:idx + 2], pb[:, :], AX.X,
                                                            OP.max, apply_absolute_value=True)
                                idx += 2
                    amr = per.tile([128, 1], f32, tag="amr")
                    nc.vector.tensor_reduce(amr[:, 0:1], am[:, 0:192], AX.X, OP.max)
                    amr2 = per.tile([128, 1], f32, tag="amr2")
                    nc.gpsimd.partition_all_reduce(amr2[:, :], amr[:, :], channels=128,
                                                   reduce_op=bass_isa.ReduceOp.max)
                    c3 = allreduce(amr2[0:1, :], [1, 1], "c3")
                    t = per.tile([1, 4], f32, tag="s1t")
                    nc.vector.tensor_scalar(t[:, 0:1], c3[:, 0:1], sc_qkv[:, 3:4], None, OP.mult)
                    nc.vector.tensor_scalar(t[:, 1:2], t[:, 0:1], 1e-8, INV_QMAX, OP.max, OP.mult)
                    nc.vector.reciprocal(t[:, 2:3], t[:, 1:2])
                    nc.vector.tensor_scalar(t[:, 3:4], t[:, 2:3], sc_qkv[:, 3:4], None, OP.mult)
                    nc.gpsimd.partition_broadcast(bc2[:, 3:4], t[0:1, 1:2], channels=128)  # s1
                    nc.gpsimd.partition_broadcast(bc2[:, 4:5], t[0:1, 3:4], channels=128)  # c

                # -------- P3 (pass B): n_s int8 + rowmax + rowsum --------
                rm = per.tile([128, 192], f32, tag="rm")
                rs = per.tile([128, 192], f32, tag="rs")
                with nc.named_scope("P3_passB"), tc.tile_pool(name="ph3", bufs=4) as ph3, \
                     tc.tile_pool(name="ps3", bufs=2, space="PSUM") as ps3:
                    for hp in range(3):
                        for qt in range(16):
                            for kh in range(2):
                                pa = ps3.tile([128, 1024], f32, tag="pa")
                                pb = ps3.tile([128, 1024], f32, tag="pb")
                                for k2 in range(2):
                                    kk = kh * 2 + k2
                                    nc.tensor.matmul(pa[:, k2 * 512:(k2 + 1) * 512],
                                                     nqk[0:64, hp, qt * 128:(qt + 1) * 128],
                                                     nqk[0:64, 3 + hp, kk * 512:(kk + 1) * 512],
                                                     start=True, stop=True,
                                                     tile_position=(0, 0), skip_group_check=True)
                                    nc.tensor.matmul(pb[:, k2 * 512:(k2 + 1) * 512],
                                                     nqk[64:128, hp, qt * 128:(qt + 1) * 128],
                                                     nqk[64:128, 3 + hp, kk * 512:(kk + 1) * 512],
                                                     start=True, stop=True,
                                                     tile_position=(64, 0), skip_group_check=True)
                                for hd, pt_ in ((0, pa), (1, pb)):
                                    hloc = 2 * hp + hd
                                    slot = kh * 96 + hloc * 16 + qt
                                    ns_t = ph3.tile([128, 1024], i8, tag="ns")
                                    nc.vector.tensor_scalar(ns_t[:, :], pt_[:, :], bc2[:, 4:5],
                                                            None, OP.mult, OP.max,
                                                            accum_out=rm[:, slot:slot + 1])
                                    pe_t = ph3.tile([128, 1024], f32, tag="pe")
                                    nc.scalar.activation(pe_t[:, :], ns_t[:, :], AF.Exp,
                                                         bias=0.0, scale=bc2[:, 3:4],
                                                         accum_out=rs[:, slot:slot + 1])
                                    nc.sync.dma_start(out=ns_dram[hp, hd, qt, kh], in_=ns_t[:, :])

                # combine halves, m2, s2, inv per row
                with tc.tile_pool(name="ph3b", bufs=1) as ph3b:
                    nc.vector.tensor_tensor(rowsum[:, :], rs[:, 0:96], rs[:, 96:192], OP.add)
                    rmax = ph3b.tile([128, 96], f32)
                    nc.vector.tensor_tensor(rmax[:, :], rm[:, 0:96], rm[:, 96:192], OP.max)
                    nrm = ph3b.tile([128, 96], i8)
                    nc.vector.tensor_scalar(nrm[:, :], rmax[:, :], 0.0, None, OP.add)
                    erm = ph3b.tile([128, 96], f32)
                    nc.scalar.activation(erm[:, :], nrm[:, :], AF.Exp, bias=0.0, scale=bc2[:, 3:4])
                    rrs = ph3b.tile([128, 96], f32)
                    nc.vector.reciprocal(rrs[:, :], rowsum[:, :])
                    amax = ph3b.tile([128, 96], f32)
                    nc.vector.tensor_tensor(amax[:, :], erm[:, :], rrs[:, :], OP.mult)
                    am1 = ph3b.tile([128, 1], f32)
                    nc.vector.tensor_reduce(am1[:, 0:1], amax[:, :], AX.X, OP.max)
                    am2 = ph3b.tile([128, 1], f32)
                    nc.gpsimd.partition_all_reduce(am2[:, :], am1[:, :], channels=128,
                                                   reduce_op=bass_isa.ReduceOp.max)
                    c4 = allreduce(am2[0:1, :], [1, 1], "c4")
                    nc.vector.tensor_scalar(s2t[:, 0:1], c4[:, 0:1], 1e-8, INV_QMAX, OP.max, OP.mult)
                    s2b = per.tile([128, 1], f32, tag="s2b")
                    nc.gpsimd.partition_broadcast(s2b[:, :], s2t[0:1, 0:1], channels=128)
                    t96 = ph3b.tile([128, 96], f32)
                    nc.vector.tensor_scalar(t96[:, :], rowsum[:, :], s2b[:, 0:1], None, OP.mult)
                    nc.vector.reciprocal(inv96[:, :], t96[:, :])

            # -------- P4 (pass C): attn quant + transpose + ctx --------
            cm = per.tile([128, 4], f32, tag="cm")
            with nc.named_scope("P4_passC"), tc.tile_pool(name="ph4", bufs=4) as ph4, \
                 tc.tile_pool(name="ph4t", bufs=6) as ph4t, \
                 tc.tile_pool(name="ps4", bufs=2, space="PSUM") as ps4:
                for hp in range(3):
                    cps = ps4.tile([128, S], f32, tag="ctx")
                    for hd in range(2):
                        hloc = 2 * hp + hd
                        for qg in range(4):
                            nats = []
                            for qi in range(4):
                                qt = qg * 4 + qi
                                ns_t = ph4.tile([128, 2, 1024], i8, tag="nsr", bufs=3)
                                nc.sync.dma_start(out=ns_t[:, :, :],
                                                  in_=ns_dram[hp, hd, qt].rearrange("a p f -> p a f"))
                                pe_t = ph4.tile([128, S], f32, tag="pe4", bufs=3)
                                nc.scalar.activation(pe_t[:, :],
                                                     ns_t[:, :, :].rearrange("p a f -> p (a f)"),
                                                     AF.Exp, bias=0.0, scale=bc2[:, 3:4])
                                na8 = ph4.tile([128, S], i8, tag="na8", bufs=3)
                                slot = hloc * 16 + qt
                                nc.vector.tensor_scalar(na8[:, :], pe_t[:, :],
                                                        inv96[:, slot:slot + 1], None, OP.mult)
                                nab = ph4.tile([128, S], bf16, tag="nab", bufs=4)
                                if qt % 2 == 0:
                                    nc.scalar.copy(nab[:, :], na8[:, :])
                                else:
                                    nc.vector.tensor_copy(nab[:, :], na8[:, :])
                                nat = ph4t.tile([128, 16, 128], bf16, tag="nat", bufs=10)
                                nc.sync.dma_start_transpose(nat[:, :, :], nab[:, :])
                                nats.append((qt, nat))
                            for kc in range(16):
                                for qt, nat in nats:
                                    nc.tensor.matmul(
                                        cps[hd * 64:(hd + 1) * 64, qt * 128:(qt + 1) * 128],
                                        nv[hp][:, kc, hd * 64:(hd + 1) * 64],
                                        nat[:, kc, :],
                                        start=(kc == 0), stop=(kc == 15),
                                        tile_position=(0, hd * 64), skip_group_check=True)
                    nc.vector.tensor_reduce(cm[:, hp:hp + 1], cps[:, :], AX.X, OP.max,
                                            apply_absolute_value=True)
                    nc.scalar.copy(ctxf[hp][:, :], cps[:, :])
                cmr = per.tile([128, 1], f32, tag="cmr")
                nc.vector.tensor_reduce(cmr[:, 0:1], cm[:, 0:3], AX.X, OP.max)
                cmr2 = per.tile([128, 1], f32, tag="cmr2")
                nc.gpsimd.partition_all_reduce(cmr2[:, :], cmr[:, :], channels=128,
                                               reduce_op=bass_isa.ReduceOp.max)
                c5 = allreduce(cmr2[0:1, :], [1, 1], "c5")

            # quantize ctx -> int8 and ship via pair AllGather
            with nc.named_scope("P5_exchange"), tc.tile_pool(name="ph5", bufs=2) as ph5, \
                 tc.tile_pool(name="dram5", bufs=1, space="DRAM") as dram5:
                nc.vector.tensor_scalar(sct[:, 0:1], s2t[:, 0:1], sc_qkv[:, 2:3], None, OP.mult)
                nc.vector.tensor_scalar(sct[:, 1:2], c5[:, 0:1], sct[:, 0:1], None, OP.mult)
                nc.vector.tensor_scalar(sct[:, 2:3], sct[:, 1:2], 1e-8, INV_QMAX, OP.max, OP.mult)
                rcc = per.tile([1, 2], f32, tag="rcc")
                nc.vector.reciprocal(rcc[:, 0:1], sct[:, 2:3])
                nc.vector.tensor_scalar(sct[:, 3:4], rcc[:, 0:1], sct[:, 0:1], None, OP.mult)  # c2
                nc.vector.tensor_scalar(sct[:, 4:5], sct[:, 2:3], sc3[:, 1:2], None, OP.mult)  # gy
                bc5 = per.tile([128, 1], f32, tag="bc5")
                nc.gpsimd.partition_broadcast(bc5[:, :], sct[0:1, 3:4], channels=128)
                gin = dram5.tile([3, 128, S], i8)
                for hp in range(3):
                    q8 = ph5.tile([128, S], i8, tag="q8")
                    nc.vector.tensor_scalar(q8[:, :], ctxf[hp][:, :], bc5[:, 0:1], None, OP.mult)
                    nc.gpsimd.dma_start(gin[hp], q8[:, :])
                gout = dram5.tile([2, 3, 128, S], i8)
                nc.gpsimd.collective_compute("AllGather", OP.bypass, replica_groups=RG_PAIR,
                                             ins=[gin.opt()], outs=[gout.opt()])
                nctx = phO.tile([128, 6, S], bf16, tag="nctx")
                for half in range(2):
                    for hp in range(3):
                        t8 = ph5.tile([128, S], i8, tag="t8")
                        nc.sync.dma_start(out=t8[:, :], in_=gout[half, hp])
                        nc.vector.tensor_copy(nctx[:, half * 3 + hp, :], t8[:, :])

            phV_cm.__exit__(None, None, None)

            # ============== P5: output projection + residual + BN ==============
            with nc.named_scope("P6_out"), tc.tile_pool(name="ph6", bufs=2) as ph6, \
                 tc.tile_pool(name="ph6r", bufs=1) as ph6r, \
                 tc.tile_pool(name="ps6", bufs=2, space="PSUM") as ps6, \
                 tc.tile_pool(name="ps6b", bufs=1, space="PSUM") as ps6b:
                ones = ph6r.tile([128, 1], f32)
                nc.vector.memset(ones[:, :], 1.0)
                x8a = ph6r.tile([128, 16, D], i8)
                for st in range(16):
                    xt_ = ph6.tile([128, D], f32, tag="xt")
                    nc.sync.dma_start(out=xt_[:, :], in_=xr_d[:, st, :])
                    nc.vector.tensor_scalar(x8a[:, st, :], xt_[:, :], bc1[:, 3:4], None, OP.mult)
                ym = per.tile([128, 16], f32, tag="ym")
                rtile = ph6r.tile([128, 16, D], f32)
                sumr = ps6b.tile([1, 1024], f32, tag="sr1")
                sumr2 = ps6b.tile([1, 1024], f32, tag="sr2")
                for st in range(16):
                    yp = ps6.tile([128, D], f32, tag="yp")
                    for dc in range(6):
                        nc.tensor.matmul(yp[:, 0:512], nctx[:, dc, st * 128:(st + 1) * 128],
                                         nwo[:, dc, 0:512], start=(dc == 0), stop=(dc == 5))
                        nc.tensor.matmul(yp[:, 512:768], nctx[:, dc, st * 128:(st + 1) * 128],
                                         nwo[:, dc, 512:768], start=(dc == 0), stop=(dc == 5))
                    nc.vector.tensor_reduce(ym[:, st:st + 1], yp[:, :], AX.X, OP.max,
                                            apply_absolute_value=True)
                    nc.scalar.copy(rtile[:, st, :], yp[:, :])
                ymr = per.tile([128, 1], f32, tag="ymr")
                nc.vector.tensor_reduce(ymr[:, 0:1], ym[:, :], AX.X, OP.max)
                ymr2 = per.tile([128, 1], f32, tag="ymr2")
                nc.gpsimd.partition_all_reduce(ymr2[:, :], ymr[:, :], channels=128,
                                               reduce_op=bass_isa.ReduceOp.max)
                c6 = allreduce(ymr2[0:1, :], [1, 1], "c6")
                syt = per.tile([1, 4], f32, tag="syt")
                nc.vector.tensor_scalar(syt[:, 0:1], c6[:, 0:1], sct[:, 4:5], None, OP.mult)
                nc.vector.tensor_scalar(syt[:, 1:2], syt[:, 0:1], 1e-8, INV_QMAX, OP.max, OP.mult)
                nc.vector.reciprocal(syt[:, 2:3], syt[:, 1:2])
                nc.vector.tensor_scalar(syt[:, 3:4], syt[:, 2:3], sct[:, 4:5], None, OP.mult)
                bc6 = per.tile([128, 2], f32, tag="bc6")
                nc.gpsimd.partition_broadcast(bc6[:, 0:1], syt[0:1, 3:4], channels=128)  # cgy
                nc.gpsimd.partition_broadcast(bc6[:, 1:2], syt[0:1, 1:2], channels=128)  # s_y
                for st in range(16):
                    fqx = ph6.tile([128, D], f32, tag="fqx")
                    nc.vector.tensor_scalar(fqx[:, :], x8a[:, st, :], bc1[:, 2:3], None, OP.mult)
                    y8 = ph6.tile([128, D], i8, tag="y8")
                    nc.vector.tensor_scalar(y8[:, :], rtile[:, st, :], bc6[:, 0:1], None, OP.mult)
                    fqy = ph6.tile([128, D], f32, tag="fqy")
                    nc.vector.tensor_scalar(fqy[:, :], y8[:, :], bc6[:, 1:2], None, OP.mult)
                    nc.vector.tensor_tensor(rtile[:, st, :], fqy[:, :], fqx[:, :], OP.add)
                    nc.tensor.matmul(sumr[:, 0:512], ones[:, :], rtile[:, st, 0:512],
                                     start=(st == 0), stop=(st == 15))
                    nc.tensor.matmul(sumr[:, 512:768], ones[:, :], rtile[:, st, 512:768],
                                     start=(st == 0), stop=(st == 15))
                    r2 = ph6.tile([128, D], f32, tag="r2")
                    nc.scalar.square(r2[:, :], rtile[:, st, :])
                    nc.tensor.matmul(sumr2[:, 0:512], ones[:, :], r2[:, 0:512],
                                     start=(st == 0), stop=(st == 15))
                    nc.tensor.matmul(sumr2[:, 512:768], ones[:, :], r2[:, 512:768],
                                     start=(st == 0), stop=(st == 15))
                sums = ph6r.tile([1, 2 * D], f32)
                nc.vector.tensor_copy(sums[:, 0:768], sumr[:, 0:768])
                nc.vector.tensor_copy(sums[:, 768:1536], sumr2[:, 0:768])
                c7 = allreduce(sums[:, :], [1, 2 * D], "c7", op=OP.add)
                mean = ph6r.tile([1, D], f32)
                nc.vector.tensor_scalar(mean[:, :], c7[:, 0:768], N_INV, None, OP.mult)
                msq = ph6r.tile([1, D], f32)
                nc.vector.tensor_tensor(msq[:, :], mean[:, :], mean[:, :], OP.mult)
                var0 = ph6r.tile([1, D], f32)
                nc.vector.tensor_scalar(var0[:, :], c7[:, 768:1536], N_INV, None, OP.mult)
                var = ph6r.tile([1, D], f32)
                nc.vector.tensor_tensor(var[:, :], var0[:, :], msq[:, :], OP.subtract)
                vare = ph6r.tile([1, D], f32)
                nc.vector.tensor_scalar(vare[:, :], var[:, :], BN_EPS, None, OP.add)
                sd = ph6r.tile([1, D], f32)
                nc.scalar.activation(sd[:, :], vare[:, :], AF.Sqrt, bias=0.0, scale=1.0)
                invstd = ph6r.tile([1, D], f32)
                nc.vector.reciprocal(invstd[:, :], sd[:, :])
                mm = ph6r.tile([1, D], f32)
                nc.vector.tensor_tensor(mm[:, :], mean[:, :], invstd[:, :], OP.mult)
                invstdB = ph6r.tile([128, D], f32)
                nc.gpsimd.partition_broadcast(invstdB[:, :], invstd[0:1, :], channels=128)
                mmB = ph6r.tile([128, D], f32)
                nc.gpsimd.partition_broadcast(mmB[:, :], mm[0:1, :], channels=128)
                rnm = per.tile([128, 16], f32, tag="rnm")
                for st in range(16):
                    t_ = ph6.tile([128, D], f32, tag="rn_t")
                    nc.vector.tensor_tensor(t_[:, :], rtile[:, st, :], invstdB[:, :], OP.mult)
                    nc.vector.tensor_tensor(rtile[:, st, :], t_[:, :], mmB[:, :], OP.subtract)
                    nc.vector.tensor_reduce(rnm[:, st:st + 1], rtile[:, st, :], AX.X, OP.max,
                                            apply_absolute_value=True)
                rnr = per.tile([128, 1], f32, tag="rnr")
                nc.vector.tensor_reduce(rnr[:, 0:1], rnm[:, :], AX.X, OP.max)
                rnr2 = per.tile([128, 1], f32, tag="rnr2")
                nc.gpsimd.partition_all_reduce(rnr2[:, :], rnr[:, :], channels=128,
                                               reduce_op=bass_isa.ReduceOp.max)
                c8 = allreduce(rnr2[0:1, :], [1, 1], "c8")
                srt = per.tile([1, 2], f32, tag="srt")
                nc.vector.tensor_scalar(srt[:, 0:1], c8[:, 0:1], 1e-8, INV_QMAX, OP.max, OP.mult)
                nc.vector.reciprocal(srt[:, 1:2], srt[:, 0:1])
                bc8 = per.tile([128, 2], f32, tag="bc8")
                nc.gpsimd.partition_broadcast(bc8[:, 0:1], srt[0:1, 1:2], channels=128)
                nc.gpsimd.partition_broadcast(bc8[:, 1:2], srt[0:1, 0:1], channels=128)
                for st in range(16):
                    o8 = ph6.tile([128, D], i8, tag="o8")
                    nc.vector.tensor_scalar(o8[:, :], rtile[:, st, :], bc8[:, 0:1], None, OP.mult)
                    of = ph6.tile([128, D], f32, tag="of")
                    nc.vector.tensor_scalar(of[:, :], o8[:, :], bc8[:, 1:2], None, OP.mult)
                    nc.sync.dma_start(out=out_d[:, st, :], in_=of[:, :])

        phO_cm.__exit__(None, None, None)
    nc.finalize()
    return nc


def _prep_inputs(x, w_in, w_out):
    """Host-side sharding (data movement only)."""
    ins = []
    woT = np.ascontiguousarray(w_out.T)
    for c in range(NCORES):
        b, hs = c // 2, c % 2
        heads = list(range(6 * hs, 6 * hs + 6))
        rows = []
        for base in (0, D, 2 * D):
            for h in heads:
                rows.append(w_in[base + h * DH: base + (h + 1) * DH])
        w_sel = np.concatenate(rows, axis=0)            # [1152, 768]
        ins.append({
            "xT": np.ascontiguousarray(x[b].T),
            "xr": np.ascontiguousarray(x[b].reshape(16, 128, D).transpose(1, 0, 2)),
            "wqkvT": np.ascontiguousarray(w_sel.T),
            "woT": woT,
        })
    return ins


def kernel(x, w_in, w_out):
    from concourse import bass2jax
    if "nc" not in _CACHE:
        _CACHE["nc"] = _build()
    nc = _CACHE["nc"]
    ins = _prep_inputs(np.asarray(x, np.float32), np.asarray(w_in, np.float32),
                       np.asarray(w_out, np.float32))
    res = bass2jax.run_bass_via_pjrt(nc, ins, n_cores=NCORES)
    out = np.empty((B, S, D), np.float32)
    for c in range(NCORES):
        b, hs = c // 2, c % 2
        full = res[c]["out"].transpose(1, 0, 2).reshape(S, D)
        out[b, hs * 1024:(hs + 1) * 1024] = full[hs * 1024:(hs + 1) * 1024]
    return out
